# revision 22
# baseline (speedup 1.0000x reference)
"""MoE feed-forward (top-2 of 8 experts) on 8 Trainium2 NeuronCores.

Strategy: expert-parallel with load balancing. Each of the 8 cores owns one
expert's weights (its "primary" set) plus optionally a second expert's
weights (its "secondary" set). The (tiny) gate runs on host as part of input
sharding: top-2 routing is computed in float64 (ordering verified robust:
min weight gap between rank-2/rank-3 experts is ~6.6e-6, far above f32
rounding noise). Tokens are gathered per expert; each expert's first C_A
tokens go to its own core, and overflow tokens (experts loaded above C_A)
are packed into 128-token blocks dispatched to cores with spare capacity,
which receive that expert's weights as their secondary set. C_A is chosen
as the smallest value whose overflow fits one 128-token block per core, so
per-core capacity C = C_A + 128 ~= max-expert-count rather than its 128-
round-up: both matmuls stream exact token counts (tokens are the moving
operand in both mm1 and mm2), so capacity is not quantized to 128.

Each core computes, in bf16 with f32 PSUM accumulation,

    y_row = silu(x_row @ W1[set]) @ W2[set]

for its gathered tokens; outputs are stored [D, C] (tokens on the moving
axis). The host then un-shards: every token's output is the combine-weight
sum of its two expert rows (weights applied on host in f64->f32).
"""

import numpy as np
import ml_dtypes

B, T, D, H, E = 4, 2048, 1024, 2048, 8
TOP_K = 2
N = B * T
P = 128
NCORES = 8
MM_FREE = 512  # PSUM bank-limited matmul free dim (fp32 out)

_compiled = {}


def _chunks(lo, hi, step):
    out = []
    while lo < hi:
        out.append((lo, min(step, hi - lo)))
        lo += min(step, hi - lo)
    return out


def _build(C_A, C_B):
    """Compile the per-core program: C_B secondary-expert tokens (may be 0)
    at layout offset 0, then C_A primary-expert tokens. C_A need not be a
    multiple of 128: both matmuls stream exact token counts."""
    import concourse.bacc as bacc
    import concourse.mybir as mybir
    import concourse.tile as tile

    fp32 = mybir.dt.float32
    bf16 = mybir.dt.bfloat16

    C = C_A + C_B
    n_sets = 2 if C_B else 1

    nc = bacc.Bacc("TRN2", target_bir_lowering=False, debug=False)

    KD = D // P   # contraction tiles for x @ W1
    KH = H // P   # contraction tiles for h @ W2
    NW1C = H // MM_FREE  # w1 column chunks

    # contraction-tiled [P, K, free] DRAM layouts (plain host reshape)
    xT = nc.dram_tensor("xT", [P, KD, C], bf16, kind="ExternalInput").ap()
    w1d = [nc.dram_tensor(f"w1{s}", [P, KD, H], bf16, kind="ExternalInput").ap()
           for s in range(n_sets)]
    w2d = [nc.dram_tensor(f"w2{s}", [P, KH, D], bf16, kind="ExternalInput").ap()
           for s in range(n_sets)]
    out = nc.dram_tensor("out", [D, C], fp32, kind="ExternalOutput").ap()
    # the last (secondary) group's output goes to a [P, KD, C_B] tensor so
    # all 8 of its d-tiles ship as ONE descriptor: 8 serialized ~0.6us
    # issues would otherwise sit on the critical tail after the last matmul
    outs = (nc.dram_tensor("outs", [P, D // P, C_B], fp32,
                           kind="ExternalOutput").ap() if C_B else None)

    # Token groups never straddle the primary/secondary boundary; layout is
    # [secondary | primary]. Processing order keeps primary first (its
    # weights DMA first). The first one (or two) primary 512-chunks form a
    # "wide" group whose h-slices are computed for both 512-token subgroups
    # before moving to the next w1 column chunk: this halves the w1 delivery
    # rate the PE demands during the DMA ramp.
    prim = _chunks(0, C_A, MM_FREE)
    wide = 2 if len(prim) >= 2 else 1
    # groups: (layout_offset, size, weight_set, subchunks)
    groups = [(C_B + prim[0][0], sum(gs for _, gs in prim[:wide]), 0,
               [(C_B + g0, gs) for g0, gs in prim[:wide]])]
    groups += [(C_B + g0, gs, 0, [(C_B + g0, gs)]) for g0, gs in prim[wide:]]
    groups += [(g0, gs, 1, [(g0, gs)]) for g0, gs in _chunks(0, C_B, MM_FREE)]

    with tile.TileContext(nc) as tc:
        with (
            tc.tile_pool(name="persist", bufs=1) as persist,
            tc.tile_pool(name="hpool", bufs=2 * KH + 2) as hpool,
            tc.tile_pool(name="opool", bufs=4) as opool,
            tc.tile_pool(name="psum1", bufs=4, space="PSUM") as psum1,
            tc.tile_pool(name="psum2", bufs=4, space="PSUM") as psum2,
        ):
            # PE warm-up: dummy matmuls on a zeroed tile, no DMA deps, so
            # they run during the initial DMA ramp. ~3.4us of PE activity
            # flips the HAM clock gate to 8/8 before the real matmuls start,
            # which otherwise run at 1.2 GHz for their first ~3.4us.
            warm = persist.tile([P, P], bf16, tag="warm", name="warm")
            nc.gpsimd.memset(warm, 0)
            for wi in range(16):
                wps = psum1.tile([P, MM_FREE], fp32, tag="ps1", name=f"wps_{wi}")
                nc.tensor.matmul(
                    wps[:64, :P], warm[:, :64], warm, start=True, stop=True
                )

            def out_dma(out_, in_):
                nc.sync.dma_start(out=out_, in_=in_)

            # ---- input loads ----
            # Per-128-row-chunk descriptors (fine-grained semaphores: each
            # matmul waits only for its own 128KB chunk; whole-tile
            # descriptors measured to cause multi-us PE gaps at the ramp).
            # Issues alternate across the Sync and Scalar HWDGE queues in PE
            # consumption order; aggregate delivery is ~310 GB/s.
            xT_sb = {}   # (gi, sub, k) -> [P, gs] tile

            def load_xT(gi, sub, k, eng):
                s0, ss = groups[gi][3][sub]
                t = persist.tile([P, ss], bf16, tag=f"xT_{gi}_{sub}_{k}",
                                 name=f"xT_{gi}_{sub}_{k}")
                eng.dma_start(out=t, in_=xT[:, k, s0:s0 + ss])
                xT_sb[(gi, sub, k)] = t

            w1_sb = [[[None] * NW1C for _ in range(KD)] for _ in range(n_sets)]
            w2_sb = [[None] * KH for _ in range(n_sets)]

            def load_w1(s, k, c, eng):
                t = persist.tile([P, MM_FREE], bf16, tag=f"w1_{s}_{k}_{c}",
                                 name=f"w1_{s}_{k}_{c}")
                eng.dma_start(
                    out=t, in_=w1d[s][:, k, c * MM_FREE:(c + 1) * MM_FREE])
                w1_sb[s][k][c] = t

            def load_w2(s, i, eng):
                t = persist.tile([P, D], bf16, tag=f"w2_{s}_{i}",
                                 name=f"w2_{s}_{i}")
                eng.dma_start(out=t, in_=w2d[s][:, i, :])
                w2_sb[s][i] = t

            # Critical ramp: (xT wide-sub0 on Scalar | w1 c0 on Sync) pairs.
            # EVERYTHING else stays on Sync: the Scalar engine also runs the
            # silus, and bulk DMA issues queued ahead of them starve the
            # psum pool (measured as a 55us PE stall). Sync-only bulk issue
            # keeps up because the wide first group's c-chunk-major order
            # needs a new w1 column chunk only every ~7us.
            for k in range(KD):
                load_xT(0, 0, k, nc.scalar)
                load_w1(0, k, 0, nc.sync)
            nsub0 = len(groups[0][3])
            for k in range(KD):
                if nsub0 > 1:
                    load_xT(0, 1, k, nc.scalar)
                load_w1(0, k, 1, nc.sync)
            for c in range(2, NW1C):
                for k in range(KD):
                    load_w1(0, k, c, nc.sync)
            gx = []  # remaining (gi, sub, k) loads
            for gi in range(1, len(groups)):
                for sub in range(len(groups[gi][3])):
                    for k in range(KD):
                        gx.append((gi, sub, k))
            for i in range(KH):
                load_w2(0, i, nc.sync)
                for _ in range(2):
                    if gx:
                        load_xT(*gx.pop(0), nc.sync)
            while gx:
                load_xT(*gx.pop(0), nc.sync)
            if n_sets > 1:
                for c in range(NW1C):
                    for k in range(KD):
                        load_w1(1, k, c, nc.sync)
                for i in range(KH):
                    load_w2(1, i, nc.sync)

            # ---- compute ----
            # mm1 produces, per 512-token subgroup, 16 h-slice tiles
            # [128, tokens] (bf16 after silu). mm2 streams tokens: for each
            # 128-row output d-tile, accumulate over the 16 h-slices with
            # W2[d-tile] stationary. Both matmuls' cost is proportional to
            # the exact token count. Combine weights are applied on host.
            def mm1(gi):
                # c-chunk-major: all h-slices of one w1 column chunk, for
                # every subgroup, before touching the next chunk — during
                # the ramp the PE demands a new 0.5MB w1 chunk only every
                # (4 * n_subs * 1.7)us, which Sync-issued DMA sustains.
                g0, gs, s, subs = groups[gi]
                hts_by_sub = [[None] * KH for _ in subs]
                for ci in range(NW1C):
                    for sub, (s0, ss) in enumerate(subs):
                        for ii in range(MM_FREE // P):
                            i = ci * (MM_FREE // P) + ii
                            ps = psum1.tile([P, MM_FREE], fp32, tag="ps1",
                                            name=f"ps1_{s0}_{i}")
                            for k in range(KD):
                                nc.tensor.matmul(
                                    ps[:, :ss],
                                    w1_sb[s][k][ci][:, ii * P:(ii + 1) * P],
                                    xT_sb[(gi, sub, k)],
                                    start=(k == 0),
                                    stop=(k == KD - 1),
                                )
                            ht = hpool.tile([P, MM_FREE], bf16, tag="hT",
                                            name=f"hT_{s0}_{i}")
                            nc.scalar.activation(
                                ht[:, :ss], ps[:, :ss],
                                mybir.ActivationFunctionType.Silu,
                            )
                            hts_by_sub[sub][i] = ht
                return [(s0, ss, s, hts_by_sub[sub])
                        for sub, (s0, ss) in enumerate(subs)]

            def mm2(unit):
                s0, ss, s, hts = unit
                batched = s == 1 and outs is not None  # secondary: one DMA
                if batched:
                    otb = persist.tile([P, KD * ss], fp32, tag="otb",
                                       name="otb")
                for j in range(KD):  # 8 output d-tiles
                    ps2 = psum2.tile([P, MM_FREE], fp32, tag="ps2",
                                     name=f"ps2_{s0}_{j}")
                    for i in range(KH):
                        nc.tensor.matmul(
                            ps2[:, :ss],
                            w2_sb[s][i][:, j * P:(j + 1) * P],
                            hts[i][:, :ss],
                            start=(i == 0),
                            stop=(i == KH - 1),
                        )
                    if batched:
                        nc.vector.tensor_scalar_mul(
                            otb[:, j * ss:(j + 1) * ss], ps2[:, :ss], 1.0)
                    else:
                        ot = opool.tile([P, MM_FREE], fp32, tag="ot",
                                        name=f"ot_{s0}_{j}")
                        nc.vector.tensor_scalar_mul(ot[:, :ss], ps2[:, :ss], 1.0)
                        out_dma(out[j * P:(j + 1) * P, s0:s0 + ss], ot[:, :ss])
                if batched:
                    out_dma(outs[:, :, :], otb)

            # software-pipelined schedule: keep one unit pending so the PE
            # always has independent mm1 work at every mm1->mm2 boundary.
            pending = list(mm1(0))
            mm2(pending.pop(0))
            for gi in range(1, len(groups)):
                units = mm1(gi)
                while pending:
                    mm2(pending.pop(0))
                pending = list(units)
            while pending:
                mm2(pending.pop(0))

    nc.compile()
    return nc


def _get_compiled(C_A, C_B):
    key = (C_A, C_B)
    if key not in _compiled:
        _compiled[key] = _build(C_A, C_B)
    return _compiled[key]


def _plan_capacity(counts):
    """Pick (C_A, C_B) minimizing total capacity C = C_A + C_B: either no
    secondary (C_A = max count) or the smallest C_A whose overflow fits in
    one 128-token secondary block per core (<= NCORES blocks total)."""
    max_cnt = int(counts.max())
    best = (max_cnt, 0)
    for C_A in range(max_cnt - 1, max(0, max_cnt - NCORES * P) - 1, -1):
        over = np.maximum(counts - C_A, 0)
        nblocks = int(np.sum(-(-over // P)))
        if nblocks > NCORES:
            break
        if C_A + P < sum(best):
            best = (C_A, P)
    return best


def kernel(**inputs):
    x = np.asarray(inputs["x"], dtype=np.float32)
    Wg = np.asarray(inputs["Wg"], dtype=np.float32)
    W1 = np.asarray(inputs["W1"], dtype=np.float32)
    W2 = np.asarray(inputs["W2"], dtype=np.float32)
    xf = np.ascontiguousarray(x.reshape(-1, D))

    # --- host-side gate + top-2 routing (float64; ordering matches f32 ref) ---
    logits = xf.astype(np.float64) @ Wg.astype(np.float64)
    w = np.exp(logits - logits.max(axis=-1, keepdims=True))
    w /= w.sum(axis=-1, keepdims=True)
    order = np.argsort(-w, axis=-1, kind="stable")[:, :TOP_K]  # [N, 2] expert ids
    tw = np.take_along_axis(w, order, axis=-1)
    tw = tw / tw.sum(axis=-1, keepdims=True)  # renormalized combine weights

    counts = np.bincount(order.ravel(), minlength=E)
    C_A, C_B = _plan_capacity(counts)
    C = C_A + C_B

    nc = _get_compiled(C_A, C_B)

    # --- dispatch: primary segment per expert-owner core + overflow blocks ---
    bf = ml_dtypes.bfloat16
    tok_of = []    # per expert: token ids routed to it (ascending)
    for e in range(E):
        sel = np.nonzero((order == e).any(axis=-1))[0]
        tok_of.append(sel)

    # overflow blocks (expert, token ids), <=128 tokens each
    blocks = []
    for e in range(E):
        for b0 in range(C_A, len(tok_of[e]), P):
            blocks.append((e, tok_of[e][b0:b0 + P]))
    assert len(blocks) <= NCORES, (counts, C_A, C_B)

    def ptile(a, k):  # [k*P, n] row-major -> [P, k, n] contraction-tiled
        return np.ascontiguousarray(
            a.reshape(k, P, a.shape[1]).transpose(1, 0, 2))

    # device layout per core: [C_B secondary tokens | C_A primary tokens]
    pos = np.empty((N, TOP_K), dtype=np.int64)
    in_maps = []
    for c in range(NCORES):
        prim_tok = tok_of[c][:C_A]
        slot = (order[prim_tok, 1] == c).astype(np.int64)
        pos[prim_tok, slot] = c * C + C_B + np.arange(len(prim_tok))

        xTe = np.zeros((D, C), dtype=bf)
        xTe[:, C_B:C_B + len(prim_tok)] = xf[prim_tok].T.astype(bf)

        m = {
            "w10": ptile(np.asarray(W1[c], dtype=bf), D // P),
            "w20": ptile(np.asarray(W2[c], dtype=bf), H // P),
        }
        if C_B:
            if c < len(blocks):
                be, btok = blocks[c]
                xTe[:, :len(btok)] = xf[btok].T.astype(bf)
                bslot = (order[btok, 1] == be).astype(np.int64)
                pos[btok, bslot] = c * C + np.arange(len(btok))
                m["w11"] = ptile(np.asarray(W1[be], dtype=bf), D // P)
                m["w21"] = ptile(np.asarray(W2[be], dtype=bf), H // P)
            else:
                m["w11"] = np.zeros((P, D // P, H), dtype=bf)
                m["w21"] = np.zeros((P, H // P, D), dtype=bf)
        m["xT"] = ptile(xTe, D // P)
        in_maps.append(m)

    from concourse.bass_utils import run_bass_kernel_spmd

    # The SPMD launch reaches the 8 NeuronCores through jax/PJRT. If the
    # calling process pinned jax to CPU (e.g. to run the reference), flip to
    # the axon platform for the launch and restore afterwards.
    import jax

    flipped = False
    try:
        n_acc = len([d for d in jax.devices() if d.platform != "cpu"])
    except Exception:
        n_acc = 0
    def _clear_backends():
        try:
            import jax.extend.backend as jeb
            jeb.clear_backends()
        except Exception:
            from jax._src import xla_bridge
            xla_bridge._clear_backends()

    if n_acc < NCORES:
        prev = jax.config.jax_platforms
        jax.config.update("jax_platforms", "axon")
        _clear_backends()
        flipped = True
    try:
        res = run_bass_kernel_spmd(nc, in_maps, core_ids=list(range(NCORES)))
    finally:
        if flipped:
            jax.config.update("jax_platforms", prev)
            _clear_backends()

    # un-shard: outputs are [D, C] per core ([P, KD, C_B] "outs" carries the
    # secondary block); combine weights applied here
    Ys = []
    for c in range(NCORES):
        Yo = np.asarray(res.results[c]["out"])
        if C_B:
            Ysec = np.asarray(res.results[c]["outs"])
            Yo = np.concatenate(
                [Ysec.transpose(1, 0, 2).reshape(D, C_B), Yo[:, C_B:]], axis=1)
        Ys.append(Yo)
    Y = np.stack(Ys)
    Yt = Y.transpose(0, 2, 1).reshape(NCORES * C, D)
    outf = (Yt[pos[:, 0]] * tw[:, 0:1] + Yt[pos[:, 1]] * tw[:, 1:2])
    return outf.reshape(B, T, D).astype(np.float32)
